# revision 1
# baseline (speedup 1.0000x reference)
"""Tensor-parallel causal attention layer (B=2, S=2048, D=4096, 32 heads)
for 8 Trainium2 NeuronCores — v2.

Changes vs v1 baseline:
- Stage C (wo) interleaved per q-chunk into the attention loop (chunk-outer,
  head-inner) so the output projection pipelines inside stage B instead of
  serializing after it; wo kept resident in SBUF (loaded once).
- PSUM pools split: psA (projections + wo, 2 banks), psS (scores, 2 banks),
  psV (PV accum + transposes, 4 banks) so batch-1 projections can overlap
  batch-0 attention without bank contention.
- psum->SBUF drains in stage C moved from ScalarE to VectorE (ScalarE was
  ~50% busy on copies while gating the exp chain).
- Partial outputs written fp16 (halves output DMA; host sums in fp32).
- Startup: first pass interleaves weight/x DMA emission and defers const
  loads so the first matmul starts ~3us in instead of ~24us.
"""

import sys

for _p in ("/opt/trn_rl_repo",):
    if _p not in sys.path:
        sys.path.insert(0, _p)

import numpy as np
import ml_dtypes

D = 4096
N_HEADS = 32
HD = 128
B = 2
S = 2048
T = B * S
N_CORES = 8
HPC = N_HEADS // N_CORES  # heads per core
O = HPC * HD  # per-core projection width (512)
TC = 512  # token chunk
NCH = T // TC  # 8 chunks
NDT = D // 128  # 32 contraction tiles
ALPHA = 1.0 / float(np.sqrt(HD))

BF16 = ml_dtypes.bfloat16

_SWAP_MASK = [i ^ 1 for i in range(32)]

_CACHE = {}
VERSION = 5


def _patch_tile_drain():
    """Walrus in this container rejects a Drain carrying more than one sem
    wait ("Too many sync wait commands").  Emit one single-wait drain per
    semaphore instead — same semantics, encodable."""
    import concourse.mybir as mybir
    import concourse.tile as tile
    from concourse.vector_clock import ScopedClock

    if getattr(tile.TileContext, "_drain_patched", False):
        return

    def _drain_and_barrier(self, tick_clock, wait_clock):
        probe = mybir.InstNoOp(name=self.nc.get_next_instruction_name())
        probe.engine = mybir.EngineType.SP
        wait_clock.add_sem_waits(probe, ScopedClock({None: tick_clock.global_clock}))
        waits = list(probe.sync_info.on_wait) if probe.sync_info else []
        sem_by_num = {s.num: s for s in self.sems.allocated().values()}
        if not waits:
            self.nc.sync.drain()
        for w in waits:
            d = self.nc.sync.drain()
            d.wait_op(sem_by_num[w.id], w.wait_value, "sem-ge")
        self.nc.all_engine_barrier()
        popped = self.nc._tile_sem_poison_stack.pop()
        assert popped is self._sem_poison
        self.nc.clear_and_free_semaphores(list(self.sems.allocated().values()))
        self.nc.all_engine_barrier()

    tile.TileContext._drain_and_barrier = _drain_and_barrier
    tile.TileContext._drain_patched = True


def build_program():
    """Build the per-core Bass program (identical on every core)."""
    import concourse.bass as bass
    import concourse.mybir as mybir
    import concourse.tile as tile

    _patch_tile_drain()
    dt = mybir.dt
    f32 = dt.float32
    f16 = dt.float16
    bf = dt.bfloat16

    nc = bass.Bass("TRN2", target_bir_lowering=False, debug=False,
                   num_devices=N_CORES)

    xT = nc.dram_tensor("xT", [D, T], bf, kind="ExternalInput")
    wqT = nc.dram_tensor("wqT", [D, O], bf, kind="ExternalInput")
    wkT = nc.dram_tensor("wkT", [D, O], bf, kind="ExternalInput")
    wvT = nc.dram_tensor("wvT", [D, O], bf, kind="ExternalInput")
    woT = nc.dram_tensor("woT", [O, D], bf, kind="ExternalInput")
    cosE = nc.dram_tensor("cosE", [128, S], f16, kind="ExternalInput")
    sinE = nc.dram_tensor("sinE", [128, S], f16, kind="ExternalInput")
    masks = nc.dram_tensor("masks", [128, 4 * TC], bf, kind="ExternalInput")
    ident = nc.dram_tensor("ident", [128, 128], bf, kind="ExternalInput")
    nc.dram_tensor("vtag", [1, VERSION], mybir.dt.float32, kind="ExternalInput")
    out = nc.dram_tensor("out", [T, D], f16, kind="ExternalOutput")

    Exp = mybir.ActivationFunctionType.Exp
    mult = mybir.AluOpType.mult
    add = mybir.AluOpType.add

    with tile.TileContext(nc) as tc:
        dram = tc.alloc_tile_pool(name="dram", bufs=1, space="DRAM")
        const_p = tc.alloc_tile_pool(name="const", bufs=1)
        wbig_p = tc.alloc_tile_pool(name="wbig", bufs=9)
        wo_p = tc.alloc_tile_pool(name="wo", bufs=1)
        xt_p = tc.alloc_tile_pool(name="xt", bufs=11)
        rot_p = tc.alloc_tile_pool(name="rot", bufs=2)
        obf_p = tc.alloc_tile_pool(name="obf", bufs=3)
        att_p = tc.alloc_tile_pool(name="att", bufs=4)   # kt + vaug tags, 4 each
        qtc_p = tc.alloc_tile_pool(name="qtc", bufs=3)
        pt_p = tc.alloc_tile_pool(name="pt", bufs=20)
        sm_p = tc.alloc_tile_pool(name="sm", bufs=8)
        ctx_p = tc.alloc_tile_pool(name="ctx", bufs=2)
        osb_p = tc.alloc_tile_pool(name="osb", bufs=2)
        psA = tc.alloc_tile_pool(name="psA", bufs=4, space="PSUM")
        psS = tc.alloc_tile_pool(name="psS", bufs=2, space="PSUM")
        psV = tc.alloc_tile_pool(name="psV", bufs=2, space="PSUM")

        # ---- persistent constants (tiles alloc'd now, DMAs deferred) ----
        cos_sb = const_p.tile([128, S], f16, tag="cos")
        sin_sb = const_p.tile([128, S], f16, tag="sin")
        mask_sb = const_p.tile([128, 4 * TC], bf, tag="mask")
        ident_sb = const_p.tile([128, 128], bf, tag="ident")
        wo_sb = wo_p.tile([128, 4, D], bf, tag="wo")

        def load_consts():
            nc.sync.dma_start(out=cos_sb[:], in_=cosE[:])
            nc.sync.dma_start(out=sin_sb[:], in_=sinE[:])
            nc.sync.dma_start(out=mask_sb[:], in_=masks[:])
            nc.sync.dma_start(out=ident_sb[:], in_=ident[:])

        # ---- DRAM scratch ---------------------------------------------
        qt_d = [dram.tile([O, S], bf, tag=f"qt{b}", name=f"qt_d{b}") for b in range(B)]
        kt_d = [dram.tile([O, S], bf, tag=f"kt{b}", name=f"kt_d{b}") for b in range(B)]
        v_d = [dram.tile([S, O], bf, tag=f"v{b}", name=f"v_d{b}") for b in range(B)]

        # ---- Stage A: projections -------------------------------------
        def x_group_dma(c, g4):
            xtile4 = xt_p.tile([128, 4, TC], bf, tag="xt", name="xt4")
            nc.sync.dma_start(
                out=xtile4[:],
                in_=xT[g4 * 512:(g4 + 1) * 512,
                       c * TC:(c + 1) * TC].rearrange(
                    "(g p) t -> p g t", p=128),
            )
            return xtile4

        def proj_pass(wT_dram, kind, b, first=False):
            # weights as 8 sub-tiles; on the very first pass interleave the
            # first chunk's x loads with the weight loads so PE starts ~3us
            # in, and defer the const loads until after.
            wt = []
            x0 = [None] * (NDT // 4)
            for g in range(NDT // 4):
                wsub = wbig_p.tile([128, 4, O], bf, tag="wbig", name=f"w{g}")
                if first and g == 0:
                    # half-granularity for the very first tiles so the first
                    # matmuls' dependencies land ~1.5us sooner
                    for hh in range(2):
                        nc.sync.dma_start(
                            out=wsub[:, 2 * hh:2 * hh + 2, :],
                            in_=wT_dram[g * 512 + hh * 256:
                                        g * 512 + (hh + 1) * 256, :].rearrange(
                                "(dt p) o -> p dt o", p=128),
                        )
                        if hh == 0:
                            xtile4 = xt_p.tile([128, 4, TC], bf, tag="xt",
                                               name="xt4")
                            nc.sync.dma_start(
                                out=xtile4[:, 0:2, :],
                                in_=xT[0:256, b * 4 * TC:b * 4 * TC + TC]
                                .rearrange("(g p) t -> p g t", p=128),
                            )
                            nc.sync.dma_start(
                                out=xtile4[:, 2:4, :],
                                in_=xT[256:512, b * 4 * TC:b * 4 * TC + TC]
                                .rearrange("(g p) t -> p g t", p=128),
                            )
                            x0[0] = xtile4
                else:
                    nc.sync.dma_start(
                        out=wsub[:],
                        in_=wT_dram[g * 512:(g + 1) * 512, :].rearrange(
                            "(dt p) o -> p dt o", p=128),
                    )
                wt.append(wsub)
                if first and g > 0:
                    x0[g] = x_group_dma(b * 4, g)
            if first:
                load_consts()
            for cl in range(4):  # chunk within batch
                c = b * 4 + cl
                ps = [psA.tile([128, TC], f32, tag="psA", name=f"psa{i}") for i in range(4)]
                for g4 in range(NDT // 4):
                    if first and cl == 0:
                        xtile4 = x0[g4]
                    else:
                        xtile4 = x_group_dma(c, g4)
                    for gi in range(4):
                        dtile = g4 * 4 + gi
                        st = dtile == 0
                        sp = dtile == NDT - 1
                        if kind != "v":
                            for ot in range(4):
                                nc.tensor.matmul(
                                    ps[ot][:],
                                    lhsT=wt[g4][:, gi, ot * 128:(ot + 1) * 128],
                                    rhs=xtile4[:, gi, :],
                                    start=st, stop=sp,
                                )
                        else:
                            for j in range(4):
                                nc.tensor.matmul(
                                    ps[j][:],
                                    lhsT=xtile4[:, gi, j * 128:(j + 1) * 128],
                                    rhs=wt[g4][:, gi, :],
                                    start=st, stop=sp,
                                )
                if kind != "v":
                    dst = qt_d[b] if kind == "q" else kt_d[b]
                    for ot in range(4):
                        rcp = rot_p.tile([128, TC], f16, tag="rcp")
                        nc.scalar.copy(rcp[:], ps[ot][:])
                        shuf = rot_p.tile([128, TC], f16, tag="shuf")
                        nc.vector.stream_shuffle(shuf[:], rcp[:], _SWAP_MASK)
                        tmp = rot_p.tile([128, TC], f16, tag="tmp")
                        nc.vector.tensor_tensor(
                            tmp[:], rcp[:],
                            cos_sb[:, cl * TC:(cl + 1) * TC], mult)
                        nc.vector.tensor_tensor(
                            shuf[:], shuf[:],
                            sin_sb[:, cl * TC:(cl + 1) * TC], mult)
                        obf = obf_p.tile([128, TC], bf, tag="obf")
                        nc.vector.tensor_tensor(obf[:], tmp[:], shuf[:], add)
                        nc.sync.dma_start(
                            out=dst[ot * 128:(ot + 1) * 128, cl * TC:(cl + 1) * TC],
                            in_=obf[:],
                        )
                else:
                    for j in range(4):
                        vbf = obf_p.tile([128, O], bf, tag="obf")
                        nc.scalar.copy(vbf[:], ps[j][:])
                        nc.sync.dma_start(
                            out=v_d[b][cl * TC + j * 128:cl * TC + (j + 1) * 128, :],
                            in_=vbf[:],
                        )

        # ---- Stage B + C: attention and output projection, per chunk ---
        ctxCs = {}
        for b in range(B):
            proj_pass(wqT, "q", b, first=(b == 0))
            proj_pass(wkT, "k", b)
            # K^T resident tiles can load during the v-pass (k_d is final)
            kt_sb = []
            for h in range(HPC):
                kt_h = att_p.tile([128, S], bf, tag="kt", name=f"kt{h}")
                nc.sync.dma_start(out=kt_h[:], in_=kt_d[b][h * 128:(h + 1) * 128, :])
                kt_sb.append(kt_h)
            if b == 0:
                nc.sync.dma_start(
                    out=wo_sb[:],
                    in_=woT[:].rearrange("(ot p) m -> p ot m", p=128))
            proj_pass(wvT, "v", b)
            vaug = []
            for h in range(HPC):
                va = att_p.tile([128, S // 128, 132], bf, tag="vaug", name=f"va{h}")
                for qr in range(4):
                    nc.sync.dma_start(
                        out=va[:, qr * 4:(qr + 1) * 4, 0:128],
                        in_=v_d[b][qr * TC:(qr + 1) * TC,
                                   h * 128:(h + 1) * 128].rearrange(
                            "(kt p) o -> p kt o", p=128),
                    )
                nc.vector.memset(va[:, :, 128:129], 1.0)
                vaug.append(va)

            def wo_part(bw, cw, tl):
                # one token-tile (128 rows) of the chunk-cw output projection
                ctxC = ctxCs[(bw, cw)]
                tt = cw * 4 + tl
                for half in range(2):
                    osb = osb_p.tile([128, D // 2], f16, tag="osb")
                    for mh in range(4):
                        pso = psA.tile([128, TC], f32, tag="psA")
                        for ot in range(4):
                            nc.tensor.matmul(
                                pso[:],
                                lhsT=ctxC[:, ot, tl * 128:(tl + 1) * 128],
                                rhs=wo_sb[:, ot,
                                          half * (D // 2) + mh * TC:
                                          half * (D // 2) + (mh + 1) * TC],
                                start=(ot == 0), stop=(ot == 3),
                            )
                        nc.vector.tensor_copy(
                            osb[:, mh * TC:(mh + 1) * TC], pso[:])
                    nc.sync.dma_start(
                        out=out[bw * S + tt * 128:bw * S + (tt + 1) * 128,
                                half * (D // 2):(half + 1) * (D // 2)],
                        in_=osb[:],
                    )

            for c in range(4):
                ctxC = ctx_p.tile([128, 4, TC], bf, tag="ctx", name=f"ctx{b}_{c}")
                ctxCs[(b, c)] = ctxC
                for h in range(HPC):
                    qt_c = qtc_p.tile([128, TC], bf, tag="qtc")
                    nc.sync.dma_start(
                        out=qt_c[:],
                        in_=qt_d[b][h * 128:(h + 1) * 128, c * TC:(c + 1) * TC])
                    pts = []
                    for kt in range(4 * c + 4):
                        # diagonal tiles only produce valid columns >= q0
                        jd = kt - 4 * c
                        q0 = max(jd, 0) * 128
                        ps_s = psS.tile([128, TC], f32, tag="psS")
                        nc.tensor.matmul(
                            ps_s[:, q0:TC],
                            lhsT=kt_sb[h][:, kt * 128:(kt + 1) * 128],
                            rhs=qt_c[:, q0:TC],
                            start=True, stop=True,
                        )
                        pt = pt_p.tile([128, TC], bf, tag="pt")
                        nc.scalar.activation(pt[:, q0:TC], ps_s[:, q0:TC], Exp)
                        if jd >= 0:
                            nc.vector.tensor_tensor(
                                pt[:, q0:TC], pt[:, q0:TC],
                                mask_sb[:, jd * TC + q0:(jd + 1) * TC], mult)
                        pts.append(pt)
                    # fill the exp latency of this head's scores with one
                    # token-tile of the previous chunk's output projection
                    if c > 0:
                        wo_part(b, c - 1, h)
                    elif b > 0:
                        wo_part(b - 1, 3, h)
                    for j in range(4):
                        pv = psV.tile([128, 132], f32, tag="psV", name=f"pv{j}")
                        for kt in range(4 * c + j + 1):
                            nc.tensor.matmul(
                                pv[:, 0:129],
                                lhsT=pts[kt][:, j * 128:(j + 1) * 128],
                                rhs=vaug[h][:, kt, 0:129],
                                start=(kt == 0), stop=(kt == 4 * c + j),
                            )
                        rec = sm_p.tile([128, 1], f32, tag="rec")
                        nc.vector.reciprocal(rec[:], pv[:, 128:129])
                        ctxn = sm_p.tile([128, 128], bf, tag="ctxn")
                        nc.vector.tensor_scalar_mul(
                            ctxn[:], pv[:, 0:128], rec[:])
                        trp = psV.tile([128, 128], bf, tag="psV")
                        nc.tensor.transpose(trp[:], ctxn[:], ident_sb[:])
                        nc.vector.tensor_copy(ctxC[:, h, j * 128:(j + 1) * 128], trp[:])
            if b == B - 1:
                for tl in range(4):
                    wo_part(b, 3, tl)

        for p in reversed([dram, const_p, wbig_p, wo_p, xt_p, rot_p, obf_p,
                           att_p, qtc_p, pt_p, sm_p, ctx_p, osb_p,
                           psA, psS, psV]):
            p.release()

    _split_multi_waits(nc, mybir, max_waits=1)
    return nc


def _split_multi_waits(nc, mybir, max_waits=1):
    """Walrus codegen in this container can only encode a limited number of
    sem waits per instruction.  Hoist extra waits onto same-engine NoOps
    placed immediately before the instruction (same program point, so
    semantics are unchanged)."""
    for f in nc.m.functions:
        for bb in f.blocks:
            new = []
            for ins in bb.instructions:
                si = ins.sync_info
                if (si is not None and len(si.on_wait) > max_waits
                        and ins.engine != mybir.EngineType.Unassigned):
                    waits = list(si.on_wait)
                    extra, keep = waits[:-max_waits], waits[-max_waits:]
                    for w in extra:
                        nop = mybir.InstNoOp(
                            name=nc.get_next_instruction_name())
                        nop.engine = ins.engine
                        nop.sync_info = mybir.SyncInfo(
                            on_wait=[w], on_update=[])
                        nc.register_instruction(nop)
                        new.append(nop)
                    ins.sync_info = mybir.SyncInfo(
                        on_wait=keep, on_update=list(si.on_update))
                new.append(ins)
            bb.instructions = new


def host_prep(x, freqs_cos, freqs_sin, wq, wk, wv, wo):
    """Build the per-core input maps (host-side shard + layout prep)."""
    x = np.asarray(x, dtype=np.float32)
    xT16 = np.ascontiguousarray(
        x.reshape(T, D).T).astype(BF16)

    fc = np.asarray(freqs_cos, dtype=np.float32)
    fs = np.asarray(freqs_sin, dtype=np.float32)
    cosE = np.repeat(fc.T, 2, axis=0).astype(np.float32)  # [128, S]
    sinE = np.repeat(fs.T, 2, axis=0).astype(np.float32)
    sinE[0::2, :] *= -1.0
    cosE = np.ascontiguousarray(cosE).astype(np.float16)
    sinE = np.ascontiguousarray(sinE).astype(np.float16)

    m = np.zeros((128, 4 * TC), dtype=np.float32)
    kk = np.arange(128)[:, None]
    qq = np.arange(TC)[None, :]
    for j in range(4):
        m[:, j * TC:(j + 1) * TC] = (128 * j + kk <= qq).astype(np.float32)
    masks = m.astype(BF16)
    identity = np.eye(128, dtype=np.float32).astype(BF16)

    wq = np.asarray(wq, dtype=np.float32)
    wk = np.asarray(wk, dtype=np.float32)
    wv = np.asarray(wv, dtype=np.float32)
    wo = np.asarray(wo, dtype=np.float32)

    in_maps = []
    for c in range(N_CORES):
        rows = slice(c * O, (c + 1) * O)
        in_maps.append({
            "xT": xT16,
            "wqT": np.ascontiguousarray(wq[rows].T * ALPHA).astype(BF16),
            "wkT": np.ascontiguousarray(wk[rows].T).astype(BF16),
            "wvT": np.ascontiguousarray(wv[rows].T).astype(BF16),
            "woT": np.ascontiguousarray(wo[:, rows].T).astype(BF16),
            "cosE": cosE,
            "sinE": sinE,
            "masks": masks,
            "ident": identity,
            "vtag": np.zeros((1, VERSION), dtype=np.float32),
        })
    return in_maps


def get_cached_program():
    if "nc" not in _CACHE:
        _CACHE["nc"] = build_program()
    return _CACHE["nc"]


def kernel(x, start_pos, freqs_cos, freqs_sin, mask, wq, wk, wv, wo):
    from concourse.bass_utils import run_bass_kernel_spmd

    nc = get_cached_program()
    in_maps = host_prep(x, freqs_cos, freqs_sin, wq, wk, wv, wo)
    res = run_bass_kernel_spmd(nc, in_maps, list(range(N_CORES)))
    acc = np.zeros((T, D), dtype=np.float32)
    for c in range(N_CORES):
        acc += np.asarray(res.results[c]["out"], dtype=np.float32)
    return acc.reshape(B, S, D)


if __name__ == "__main__":
    nc = build_program()
    print("program built ok")

